# revision 37
# baseline (speedup 1.0000x reference)
"""Trainium2 Bass kernel for the binarized-conv BasicBlock problem.

Math restructure (exact up to fp32r-grade rounding):
  wb = sign(weight)
  out = clip( A * conv(x, wb) + x + B , -1, 1 )
where
  A[o]     = gamma/sqrt(var+eps) * (1 + w1[o])                (per channel)
  B[o,h,w] = bs*(conv(S1,wb) + w1*conv(S2,wb))[o,h,w] + bb[o] (batch-independent
             shift/edge field, computed on host)

PE mapping: per image pair, the K=128 contraction dim carries BOTH images'
64 channels ([imgA ch; imgB ch], raw fp32 viewed as float32r — the PE rounds
to 11 mantissa bits internally, max rel err ~1.2e-2 vs the fp32 reference).
Each tap is ONE K=128, M=128 matmul with block-diagonal weights
([[wb_t, 0], [0, wb_t]]), so both images' conv accumulates in a single
N=448 column stream: 9 taps x 7 row-chunks per pair, half the PE time of
an fp16 [x16; d16] split layout (which needs separate matmuls per image).

Epilogue per chunk: u = A*psum + x (DVE STT, full 128 partitions in one op,
raw-fp32 residual from the rhs tile), o = u + B (Pool/GPSIMD), then
clamp to [-1,1] + fp16 convert (DVE tensor_scalar) into a per-pair staging
tile; one fp16 output DMA per pair. Host upcasts to fp32.
Sharding: batch 64 -> 8 cores x 8 images (4 pairs).
"""
import sys
for _p in ('/opt/trn_rl_repo',):
    if _p not in sys.path:
        sys.path.insert(0, _p)

import numpy as np
import concourse.bass as bass
import concourse.bacc as bacc
import concourse.tile as tile
import concourse.mybir as mybir
from concourse import bass_utils

BN_EPS = 1e-5
N_CORES = 8
C, H, W = 64, 56, 56
HP, WP = H + 2, W + 2           # padded spatial
NPOS = H * W                    # 3136
PPOS = HP * WP                  # 3364
IMGS = 8                        # images per core
PAIRS = IMGS // 2
CH_ROWS = 8                     # output rows per chunk
NCHUNK = H // CH_ROWS           # 7
CHL = CH_ROWS * W               # 448
# input row-groups: (row_off, nrows); chunk c needs padded rows 8c..8c+9,
# epilogue needs rows 8c+1..8c+8 — both within the group's range. First two
# chunks get their own small groups so the pipeline starts early.
IN_GROUPS = [(0, 10), (8, 10), (16, 18), (32, 18), (48, 10)]
CHUNK_GROUP = {0: 0, 1: 1, 2: 2, 3: 2, 4: 3, 5: 3, 6: 4}

f32 = mybir.dt.float32
f32r = mybir.dt.float32r
f16 = mybir.dt.float16
ALU = mybir.AluOpType
AF = mybir.ActivationFunctionType

_CACHE = {}


def _build_module(hw_loop=0, ablate=(), compile=True, block=None):
    # block = chunks interleaved per tap-sweep. 4 measured fastest on HW:
    # distance-3 PSUM bank rotation avoids the same-bank accumulation stall
    # (~307 -> ~265 ns/matmul) while chunk epilogues still overlap the next
    # block's matmuls (full t-outer serializes pair-to-pair on the epilogue).
    if block is None:
        import os
        block = int(os.environ.get("K_BLOCK", "4"))
    nc = bacc.Bacc("TRN2", target_bir_lowering=False, debug=False,
                   enable_asserts=False, num_devices=N_CORES)

    # [pair, {imgA ch 0-63; imgB ch 64-127}, 58*58] raw fp32 (padded)
    xr_d = nc.dram_tensor("xr", [PAIRS, 128, PPOS], f32r, kind="ExternalInput").ap()
    w_d = nc.dram_tensor("wt", [128, 9 * 128], f32r, kind="ExternalInput").ap()
    a_d = nc.dram_tensor("ascale", [128, 1], f32, kind="ExternalInput").ap()
    b_d = nc.dram_tensor("bfield", [128, NPOS], f32, kind="ExternalInput").ap()
    y_d = nc.dram_tensor("y", [PAIRS, 128, NPOS], f16, kind="ExternalOutput").ap()

    with tile.TileContext(nc) as tc:
        with tc.tile_pool(name="const", bufs=1) as constp, \
             tc.tile_pool(name="rhs", bufs=2) as rhsp, \
             tc.tile_pool(name="eout", bufs=3) as outp, \
             tc.tile_pool(name="ystage", bufs=2) as ystp, \
             tc.tile_pool(name="psum", bufs=1, space="PSUM") as psp:
            # tap-0 weights in their own tile so the first matmul starts ASAP
            wt0 = constp.tile([128, 128], f32r)
            nc.sync.dma_start(wt0[:], w_d[:, 0:128])
            wtr = constp.tile([128, 8 * 128], f32r)
            nc.sync.dma_start(wtr[:], w_d[:, 128:9 * 128])
            at = constp.tile([128, 1], f32)
            nc.sync.dma_start(at[:], a_d[:])
            # B/A in 7 per-chunk tiles so chunk 0's PSUM preload doesn't wait
            # for the whole field
            bts = []
            for c in range(NCHUNK):
                btc = constp.tile([128, CHL], f32, tag=f"bt{c}", name=f"bt{c}")
                nc.sync.dma_start(btc[:], b_d[:, c * CHL:(c + 1) * CHL])
                bts.append(btc)

            # warm-up: a few dependency-free matmuls into the spare 8th PSUM
            # bank while the first input DMAs land — ramps the PE p-state
            # (0.65 -> 2.4 GHz) so real matmuls start at full clock
            dummy = constp.tile([128, CHL], f32r)
            nc.vector.memset(dummy[:].bitcast(f32), 0.0)
            psw = psp.tile([128, CHL], f32, tag="psw", name="psw")
            for _ in range(6):
                nc.tensor.matmul(psw[:], dummy[:, 0:128], dummy[:],
                                 start=True, stop=True, skip_group_check=True)

            def _body():
              for p in range(PAIRS):
                # input in 4 row-groups so early chunks start sooner
                rgs, rg3, xg3 = [], [], []
                for gi, (roff, nrows) in enumerate(IN_GROUPS):
                    rg = rhsp.tile([128, nrows * WP], f32r, tag=f"rg{gi}")
                    if 'dma_in' not in ablate:
                        nc.gpsimd.dma_start(
                            rg[:], xr_d[p][:, roff * WP:(roff + nrows) * WP])
                    rgs.append(rg)
                    rg3.append(rg[:].rearrange("p (h w) -> p h w", w=WP))
                    xg3.append(rg[:].bitcast(f32).rearrange("p (h w) -> p h w", w=WP))
                # output staged in two tiles so the bulk DMA overlaps chunk 6
                yta = ystp.tile([128, 6 * CHL], f16, tag="yta")
                ytb = ystp.tile([128, CHL], f16, tag="ytb")

                # Pool pre-folds v = x + B per chunk as soon as the input
                # lands — removes the B-add from the post-matmul tail chain
                vws = []
                for c in range(NCHUNK):
                    g = CHUNK_GROUP[c]
                    r0 = CH_ROWS * c - IN_GROUPS[g][0]
                    vw = outp.tile([128, CHL], f32, tag=f"vw{c}", name=f"vw{c}")
                    xw = xg3[g][:, r0 + 1: r0 + 1 + CH_ROWS, 1: 1 + W]
                    nc.gpsimd.tensor_tensor(vw[:], xw, bts[c][:], ALU.add)
                    vws.append(vw)

                if p == PAIRS - 1 and block == 4:
                    # last pair: [3,3,1] so less epilogue trails the final
                    # matmul (distance-2 rotation keeps the fast HW rate)
                    cblocks = [[0, 1, 2], [3, 4, 5], [6]]
                else:
                    cblocks = [list(range(b0, min(b0 + block, NCHUNK)))
                               for b0 in range(0, NCHUNK, block)]
                for cb in cblocks:
                    pss = {}
                    for c in cb:
                        pss[c] = psp.tile([128, CHL], f32, tag=f"ps{c}",
                                          name=f"ps{c}")
                    if 'matmul' not in ablate:
                        # taps outer, chunks inner: consecutive matmuls hit
                        # different PSUM banks so they pipeline on HW
                        for t in range(9):
                            k, l = divmod(t, 3)
                            lhsT = wt0[:] if t == 0 else \
                                wtr[:, (t - 1) * 128: t * 128]
                            for c in cb:
                                g = CHUNK_GROUP[c]
                                r0 = CH_ROWS * c - IN_GROUPS[g][0]
                                rhs = rg3[g][:, r0 + k: r0 + k + CH_ROWS,
                                             l: l + W]
                                nc.tensor.matmul(pss[c][:], lhsT, rhs,
                                                 start=(t == 0), stop=(t == 8),
                                                 skip_group_check=True)
                    for c in cb:
                        g = CHUNK_GROUP[c]
                        r0 = CH_ROWS * c - IN_GROUPS[g][0]
                        ytap = yta[:, c * CHL:(c + 1) * CHL] if c < 6 else ytb[:]
                        if 'epilogue' in ablate:
                            nc.vector.tensor_copy(ytap, pss[c][:])
                        else:
                            # u = A*psum + (x+B), rounded to fp16: only the
                            # clipped [-1,1] range survives, where fp16's
                            # ~5e-4 is fine — clamp then runs at 2x 16-bit
                            u = outp.tile([128, CHL], f16, tag="u")
                            nc.vector.scalar_tensor_tensor(
                                u[:], pss[c][:], at[:], vws[c][:],
                                ALU.mult, ALU.add)
                            nc.vector.tensor_scalar(ytap, u[:],
                                                    1.0, -1.0, ALU.min, ALU.max)
                if 'dma_out' not in ablate:
                    nc.sync.dma_start(y_d[p][:, 0:6 * CHL], yta[:])
                    nc.sync.dma_start(y_d[p][:, 6 * CHL:NPOS], ytb[:])

            if hw_loop:
                with tc.For_i(0, hw_loop, 1):
                    _body()
            else:
                _body()

    if compile:
        nc.compile()
    return nc


def _host_prep(x, shift1, shift2, weight, w1, gamma, beta, running_mean, running_var):
    x = np.asarray(x, np.float32)
    s1 = np.asarray(shift1, np.float32).reshape(C)
    s2 = np.asarray(shift2, np.float32).reshape(C)
    w = np.asarray(weight, np.float32)
    w1v = np.asarray(w1, np.float32).reshape(C)
    gamma = np.asarray(gamma, np.float32)
    beta = np.asarray(beta, np.float32)
    mean = np.asarray(running_mean, np.float32)
    var = np.asarray(running_var, np.float32)

    wb = np.sign(w).astype(np.float32)
    bs = (gamma / np.sqrt(var + BN_EPS)).astype(np.float32)
    A = (bs * (1.0 + w1v)).astype(np.float32)
    bb = (beta - mean * bs).astype(np.float32)

    G1 = np.einsum('oikl,i->okl', wb, s1)
    G2 = np.einsum('oikl,i->okl', wb, s2)
    G = bs[:, None, None] * (G1 + w1v[:, None, None] * G2)
    B = np.zeros((C, H, W), np.float32)
    hh = np.arange(H)[:, None]
    ww = np.arange(W)[None, :]
    for k in range(3):
        for l in range(3):
            m = ((hh + k - 1 >= 0) & (hh + k - 1 < H) &
                 (ww + l - 1 >= 0) & (ww + l - 1 < W)).astype(np.float32)
            B += G[:, k, l][:, None, None] * m[None]
    B += bb[:, None, None]

    # block-diag weights: lhsT[k, m] per tap t: [[wb_t, 0], [0, wb_t]]
    wbT = wb.transpose(1, 0, 2, 3)  # [i, o, k, l]
    wtile = np.zeros((128, 9 * 128), np.float32)
    for t in range(9):
        k, l = divmod(t, 3)
        blk = wbT[:, :, k, l]  # [i(K), o(M)]
        wtile[0:64, t * 128: t * 128 + 64] = blk
        wtile[64:128, t * 128 + 64: t * 128 + 128] = blk

    N = x.shape[0]
    xr = np.zeros((N // 2, 128, HP, WP), np.float32)
    xr[:, 0:64, 1:H + 1, 1:W + 1] = x[0::2]
    xr[:, 64:128, 1:H + 1, 1:W + 1] = x[1::2]
    xr = xr.reshape(N // 2, 128, PPOS)

    a128 = np.concatenate([A, A]).reshape(128, 1).astype(np.float32)
    b128 = np.concatenate([B.reshape(C, NPOS)] * 2, axis=0).astype(np.float32)
    return xr, wtile, a128, b128


def kernel(**inputs):
    xr, wtile, a128, b128 = _host_prep(**inputs)
    if 'nc' not in _CACHE:
        _CACHE['nc'] = _build_module()
    nc = _CACHE['nc']

    in_maps = []
    for core in range(N_CORES):
        in_maps.append({
            "xr": np.ascontiguousarray(xr[core * PAIRS:(core + 1) * PAIRS]),
            "wt": wtile,
            "ascale": a128,
            "bfield": b128,
        })
    _CACHE['in_maps'] = in_maps
    res = bass_utils.run_bass_kernel_spmd(nc, in_maps,
                                          core_ids=list(range(N_CORES)))
    _CACHE['last_result'] = res

    N = N_CORES * IMGS
    y = np.empty((N, C, H, W), np.float32)
    for core in range(N_CORES):
        yc = res.results[core]["y"].astype(np.float32)  # [PAIRS, 128, NPOS]
        yc = yc.reshape(PAIRS * 2, C, H, W)
        y[core * IMGS:(core + 1) * IMGS] = yc
    return y
